# revision 28
# baseline (speedup 1.0000x reference)
"""Causal self-attention (B=2, T=2048, C=1024, H=16) on 8 trn2 NeuronCores.

Sharding: data-parallel on batch (2 groups of 4 cores) x tensor-parallel on
heads (4 heads per core). Each core computes qkv projection for its heads,
causal attention, and a partial out-projection over its heads' slice of the
hidden dim; the host sums the 4 partials per batch.

Device layout notes:
  - x is pre-transposed on host to xT [C, T] so the C contraction sits on
    SBUF partitions for both qkv matmuls.
  - q and k are produced transposed ([64, T] per head) packed in head-pairs
    into [128, T] tiles; v is produced natural [T, 64] with an appended
    ones column so the softmax denominator falls out of the AV matmul
    (row 64 of the PSUM accumulator).
  - scores for the two heads of a pair are computed back-to-back into the
    two banks of one [128, 1024] PSUM tile; with K=64 the PE auto-row-tiles
    (tile_position (0,0)/(64,0)) so the two matmuls run concurrently and
    the second LDWEIGHTS is pulled ahead.
  - softmax runs without max subtraction (logits are O(6) for N(0,1)
    inputs, safe in fp32 exp); one full-width [128, 1024] EXP per k-tile
    covers both heads (fewer ACTIVATE fixed overheads). Columns below the
    causal diagonal may hold stale PSUM garbage; they are exp'd but never
    consumed by the AV matmuls.
  - matmuls run in float16 (1 cycle/row on the PE like bf16, but 10
    mantissa bits; all intermediate values here are O(3000) max, far from
    the fp16 range limit). PSUM accumulation is fp32.
  - phase 1 computes ONLY pair 0's q,k (f-tiles 0,1) c-major with
    1024-wide moving operands, so pair-0 attention (and the EXP stream on
    ACT, the second-busiest engine) starts as soon as the input DMAs land
    (~21us) instead of after the full projection (~45us). The v projection
    and pair-1's q,k (f2,f3) run as PE fillers inside pair-0's attention,
    keeping the PE dense (the PE clock drops 2.4->1.2 GHz for 3us after
    any idle gap, so density is also a clock win).
  - pair 1 walks q-blocks ascending with out-projection (both pairs
    accumulated in one PSUM pass, 1024-wide) queued as fillers after each
    q-block's normalize, so the out-proj work spreads across pair-1's
    exp-bound stretch instead of piling into a DVE-bound tail.
  - PSUM evacuations alternate DVE/ACT wherever ACT has slack.
"""

import sys

for _p in ("/root/.axon_site", "/root/.axon_site/_ro/trn_rl_repo", "/opt/trn_rl_repo"):
    if _p not in sys.path:
        sys.path.append(_p)

import numpy as np

B, T, C = 2, 2048, 1024
H, D = 16, 64
N_CORES = 8
HEADS_PER_CORE = H // 4  # 4 head-groups x 2 batches = 8 cores

_cache = {}
TRACE = False


def _build(T, C, nhc, D, n_cores):
    """Build + compile the SPMD program. nhc = heads per core (must be 4)."""
    import concourse.tile as tile
    from concourse import bacc, mybir

    f32 = mybir.dt.float32
    f16 = mybir.dt.float16
    EXP = mybir.ActivationFunctionType.Exp

    assert nhc % 2 == 0 and D == 64
    npairs = nhc // 2
    assert npairs == 2, "out-proj staging assumes exactly 2 head pairs"
    F = 2 * nhc * D          # qk projection output rows (q+k for nhc heads)
    VW = nhc * D             # v projection width
    CIN = nhc * D            # local c_in slice for out-proj
    NC_ = C // 128           # contraction tiles
    NF = F // 128            # qk f-tiles (= 2*npairs)
    NTT = T // 128           # 128-wide t-tiles
    QW = min(512, T)         # q-block width
    SJT = QW // 128          # 128-tiles per q-block
    NQB = T // QW            # q-blocks
    scale = 1.0 / np.sqrt(D)

    nc = bacc.Bacc("TRN2", target_bir_lowering=False, debug=False,
                   enable_asserts=False, num_devices=n_cores)

    xT = nc.dram_tensor("xT", [C, T], f16, kind="ExternalInput").ap()
    wqk = nc.dram_tensor("wqk", [C, F], f16, kind="ExternalInput").ap()
    wv = nc.dram_tensor("wv", [C, VW], f16, kind="ExternalInput").ap()
    wo = nc.dram_tensor("wo", [CIN, C], f16, kind="ExternalInput").ap()
    tri = nc.dram_tensor("tri", [128, 128], f16, kind="ExternalInput").ap()
    out = nc.dram_tensor("out", [T, C], f16, kind="ExternalOutput").ap()

    with tile.TileContext(nc) as tc:
        with tc.tile_pool(name="qk", bufs=NF) as qk_pool, \
             tc.tile_pool(name="v", bufs=NTT) as v_pool, \
             tc.tile_pool(name="misc", bufs=1) as misc_pool, \
             tc.tile_pool(name="attn2", bufs=npairs) as attn2_pool, \
             tc.tile_pool(name="wo", bufs=npairs) as wo_pool, \
             tc.tile_pool(name="obh", bufs=3) as obh_pool:

            tri_sb = misc_pool.tile([128, 128], f16, tag="tri")
            nc.sync.dma_start(tri_sb[:], tri[:])

            # HAM warm-up: the PE's activity monitor keeps the array at
            # half clock (K=4/8, 1.2 GHz) until it sees ~3.4us of sustained
            # matmul activity. A burst of matmuls fed from a memset scratch
            # tile (no DMA dependency, starts within the first microsecond)
            # trips it to full clock during the input-DMA window, so the
            # projection matmuls that follow run at 2.4 GHz instead of 1.2.
            scratch = misc_pool.tile([128, 128], f16, tag="scratch")
            nc.vector.memset(scratch[:], 1.0)
            with tc.tile_pool(name="warm", bufs=1, space="PSUM") as psW:
                pw = psW.tile([128, 128], f32, tag="warm")
                for _ in range(54):
                    nc.tensor.matmul(pw[:], scratch[:], scratch[:],
                                     start=True, stop=True)

            qk_sb = [qk_pool.tile([128, T], f16, tag="qk", name=f"qk{i}") for i in range(NF)]
            v_sb = [v_pool.tile([128, nhc * 65], f16, tag="v", name=f"v{i}") for i in range(NTT)]
            attn2_sb = [attn2_pool.tile([128, T], f16, tag="attn2",
                                        name=f"attn2_{i}") for i in range(npairs)]
            # ones columns of every v tile are constant; write them all
            # upfront (engines are idle during the DMA window) so the v
            # evacuation in the filler path is a single copy
            for tt in range(NTT):
                dstv = v_sb[tt][:].rearrange("p (h e) -> p h e", e=65)
                nc.vector.memset(dstv[:, :, 64:65], 1.0)

            def emit_v_group(tt, psp, xT_sb, wv_sb, nbufs=2):
                """Generator: one v-proj matmul per next() for t-tile tt."""
                p = psp.tile([128, 512], f32, tag="fill", bufs=nbufs,
                             name=f"fill_v_{tt}")
                for c in range(NC_):
                    nc.tensor.matmul(
                        p[:, 0:VW],
                        xT_sb[c][:, tt * 128:(tt + 1) * 128],
                        wv_sb[c][:],
                        start=(c == 0), stop=(c == NC_ - 1))
                    yield
                dst = v_sb[tt][:].rearrange("p (h e) -> p h e", e=65)
                srcp = p[:, 0:VW].rearrange("p (h e) -> p h e", e=64)
                nc.vector.tensor_copy(dst[:, :, 0:64], srcp)

            def emit_qk23_group(f, tb, psp, xT_sb, wqk_sb, nbufs=2):
                """Generator: one qk-proj matmul per next() for pair-1's
                f-tile `f`, t-block tb (512 wide)."""
                p = psp.tile([128, 512], f32, tag="fill", bufs=nbufs,
                             name=f"fill_qk_{f}_{tb}")
                for c in range(NC_):
                    nc.tensor.matmul(
                        p[:],
                        wqk_sb[c][:, f * 128:(f + 1) * 128],
                        xT_sb[c][:, tb * 512:(tb + 1) * 512],
                        start=(c == 0), stop=(c == NC_ - 1))
                    yield
                nc.vector.tensor_copy(qk_sb[f][:, tb * 512:(tb + 1) * 512],
                                      p[:])

            def emit_outproj(tt, psp):
                """Out-proj for t-tile tt: both pairs accumulate in PSUM,
                split evacuation DVE/ACT, DMA out. pr-outer so the attn2
                stationary is reused across the two column blocks. Shares
                the "fill" PSUM buffers with the v/f23 filler generators
                (all fillers flow through one FIFO, so the two banks
                cycle naturally)."""
                obh = obh_pool.tile([128, C], f16, tag="obh", bufs=4,
                                    name=f"obh_{tt}")
                ps = [psp.tile([128, 512], f32, tag="fill", bufs=2,
                               name=f"pso_{tt}_{co}")
                      for co in range(C // 512)]
                for pr in range(npairs):
                    for co in range(C // 512):
                        nc.tensor.matmul(
                            ps[co][:],
                            attn2_sb[pr][:, tt * 128:(tt + 1) * 128],
                            wo_sb[pr][:, co * 512:(co + 1) * 512],
                            start=(pr == 0), stop=(pr == npairs - 1))
                        yield
                nc.vector.tensor_copy(obh[:, 0:512], ps[0][:])
                nc.scalar.copy(obh[:, 512:1024], ps[1][:])
                nc.sync.dma_start(out[tt * 128:(tt + 1) * 128, :], obh[:])

            class FillQueue:
                """FIFO of (key, generator) PE-work, pulled between attention
                k-tile steps to keep the PE dense while ACT runs exp."""

                def __init__(self):
                    self.gens = []
                    self.nslot = 0
                    self.npulled = 0
                    self.done = set()
                    # pull() leaves this many groups untouched; drain()
                    # spends them covering the final normalize chain
                    self.reserve_groups = 0

                def add(self, gen, key=None):
                    self.gens.append((key, gen))

                def _step(self):
                    key, g = self.gens[0]
                    try:
                        next(g)
                        self.npulled += 1
                    except StopIteration:
                        self.done.add(key)
                        self.gens.pop(0)

                def pull(self, spf):
                    self.nslot += 1
                    while (len(self.gens) > self.reserve_groups
                           and self.npulled < self.nslot * spf):
                        self._step()

                def require(self, key):
                    while self.gens and key not in self.done:
                        self._step()

                def drain(self):
                    while self.gens:
                        self._step()

            def attention_pair(pair, psS, psV, r_pool, rb_pool, exp_pool,
                               queue, spf, on_qb_done=None, on_qb_start=None,
                               pull_min_jt=2, terminal_last_qb=False,
                               require_v=False):
                """Attention for both heads of `pair`, head-merged: the two
                heads' score matmuls go to the two banks of one [128, 2*QW]
                PSUM tile (concurrent via PE row tiling), one EXP covers
                both. AV for k-tile jt is emitted after the scores of jt+1;
                queue fillers cover the residual ACT latency."""
                Q2 = qk_sb[2 * pair]
                K2 = qk_sb[2 * pair + 1]
                for qb in range(NQB):
                    if on_qb_start is not None:
                        on_qb_start(qb)
                    njt = SJT * (qb + 1)
                    av = [psV.tile([65, QW], f32, tag="psav",
                                   name=f"psav_{pair}_{qb}_{hh}")
                          for hh in range(2)]

                    def emit_av(jt, ex, col0, av=av, pair=pair, njt=njt):
                        for hh in range(2):
                            h = pair * 2 + hh
                            nc.tensor.matmul(
                                av[hh][:, col0:QW],
                                v_sb[jt][:, h * 65:(h + 1) * 65],
                                ex[:, hh * QW + col0:(hh + 1) * QW],
                                start=(jt == 0), stop=(jt == njt - 1))

                    pend = None
                    for jt in range(njt):
                        d = jt - SJT * qb
                        col0 = max(d, 0) * 128
                        if require_v:
                            queue.require(("v", jt))
                        sc = psS.tile([128, 2 * QW], f32, tag="pssc",
                                      name=f"pssc_{pair}_{qb}_{jt}")
                        nc.tensor.matmul(
                            sc[:, col0:QW],
                            K2[0:64, jt * 128:(jt + 1) * 128],
                            Q2[0:64, qb * QW + col0:(qb + 1) * QW],
                            start=True, stop=True)
                        nc.tensor.matmul(
                            sc[:, QW + col0:2 * QW],
                            K2[64:128, jt * 128:(jt + 1) * 128],
                            Q2[64:128, qb * QW + col0:(qb + 1) * QW],
                            start=True, stop=True)
                        ex = exp_pool.tile([128, 2 * QW], f16, tag="ex",
                                           bufs=4,
                                           name=f"ex_{pair}_{qb}_{jt}")
                        # one call covering both heads' valid regions
                        # [col0:QW] and [QW+col0:2QW]; the gap in between
                        # is exp'd garbage that nothing reads
                        nc.scalar.activation(ex[:, col0:2 * QW],
                                             sc[:, col0:2 * QW], EXP,
                                             scale=scale)
                        if d >= 0:
                            nc.vector.tensor_mul(
                                ex[:, col0:col0 + 128],
                                ex[:, col0:col0 + 128], tri_sb[:])
                            nc.vector.tensor_mul(
                                ex[:, QW + col0:QW + col0 + 128],
                                ex[:, QW + col0:QW + col0 + 128], tri_sb[:])
                        # pair 1 pulls nothing in the first k-tiles of a
                        # q-block: freshly queued out-proj fillers depend on
                        # the previous q-block's normalize chain, and pulling
                        # them here would park them at the head of the
                        # in-order PE queue ahead of this q-block's
                        # independent score/AV matmuls. Pair 0's fillers are
                        # DMA-gated only, so it pulls everywhere.
                        if jt >= pull_min_jt:
                            queue.pull(spf)
                        if pend is not None:
                            emit_av(*pend)
                        pend = (jt, ex, col0)
                    emit_av(*pend)
                    # normalize rows 0..63 by row 64 (sum of exp): emit
                    # both heads' copy/recip chains before the
                    # broadcasts+muls so the in-order vector queue is not
                    # blocked waiting on the first gpsimd broadcast.
                    # Mid-kernel the accumulators are staged to SBUF to
                    # recycle the PSUM banks quickly; for the final
                    # q-block (nothing needs the banks after it) the chain
                    # reads PSUM directly, cutting the staging copy out of
                    # the tail's critical path.
                    terminal = terminal_last_qb and qb == NQB - 1
                    avsb2, rb2 = [], []
                    for hh in range(2):
                        if terminal:
                            src = av[hh]
                        else:
                            # stage hh0 via DVE and hh1 via ACT in parallel
                            # so both av PSUM banks free before the next
                            # q-block's first AV matmuls need them
                            src = r_pool.tile([65, QW], f32, tag=f"avs{hh}")
                            if hh == 0:
                                nc.vector.tensor_copy(src[:], av[hh][:])
                            else:
                                nc.scalar.copy(src[:], av[hh][:])
                        r1s = r_pool.tile([1, QW], f32, tag=f"r1s_{hh}")
                        nc.vector.tensor_copy(r1s[:], src[64:65, :])
                        r1 = r_pool.tile([1, QW], f32, tag=f"r1_{hh}")
                        nc.vector.reciprocal_approx_fast(r1[:], r1s[:])
                        rb = rb_pool.tile([64, QW], f32, tag=f"rb{hh}")
                        nc.gpsimd.partition_broadcast(rb[:], r1[:])
                        avsb2.append(src)
                        rb2.append(rb)
                    for hh in range(2):
                        nc.vector.tensor_mul(
                            attn2_sb[pair][hh * 64:(hh + 1) * 64,
                                           qb * QW:(qb + 1) * QW],
                            avsb2[hh][0:64, :], rb2[hh][:])
                    if on_qb_done is not None:
                        on_qb_done(qb)

            # ---- phase 1: loads + qk proj f0,f1 only ----
            with tc.tile_pool(name="xT", bufs=NC_) as xT_pool, \
                 tc.tile_pool(name="wqk", bufs=NC_) as wqk_pool, \
                 tc.tile_pool(name="wv", bufs=NC_) as wv_pool:

                xT_sb, wqk_sb, wv_sb = [], [], []
                # wv interleaves with the wqk/x stream: pair-0's first AV
                # needs v tile 0 (all wv c-tiles) only ~2us after its first
                # scores, so wv can't wait for the whole x stream anymore
                for c in range(NC_):
                    w1 = wqk_pool.tile([128, F], f16, tag="wqk")
                    nc.sync.dma_start(w1[:], wqk[c * 128:(c + 1) * 128, :])
                    wqk_sb.append(w1)
                    x1 = xT_pool.tile([128, T], f16, tag="xT")
                    nc.sync.dma_start(x1[:], xT[c * 128:(c + 1) * 128, :])
                    xT_sb.append(x1)
                    w2 = wv_pool.tile([128, VW], f16, tag="wv")
                    nc.sync.dma_start(w2[:], wv[c * 128:(c + 1) * 128, :])
                    wv_sb.append(w2)
                # weight for out-proj is needed late; load after x/qkv weights
                wo_sb = []
                for pr in range(npairs):
                    wt = wo_pool.tile([128, C], f16, tag="wo", name=f"wo{pr}")
                    nc.sync.dma_start(wt[:], wo[pr * 128:(pr + 1) * 128, :])
                    wo_sb.append(wt)

                # qk proj for pair-0's f-tiles (f0, f1) only, c-major over
                # all four 512-wide t-blocks: 8 accumulators fill PSUM and
                # each matmul runs as soon as its c-tile DMA lands. f2/f3
                # and the v projection run later as attention fillers.
                with tc.tile_pool(name="psP", bufs=8, space="PSUM") as psP:
                    ps1 = {}
                    for f in range(2):
                        for tb in range(4):
                            ps1[(f, tb)] = psP.tile(
                                [128, 512], f32, tag="p1a", bufs=8,
                                name=f"p1a_{f}_{tb}")
                    # last c-tile runs tb-major with the evacuation of each
                    # finished tb emitted immediately (DVE/ACT alternating),
                    # so the tb0 casts qb0's first scores need hide under
                    # the remaining tb1-3 matmuls
                    for c in range(NC_ - 1):
                        for f in range(2):
                            for tb in range(4):
                                nc.tensor.matmul(
                                    ps1[(f, tb)][:],
                                    wqk_sb[c][:, f * 128:(f + 1) * 128],
                                    xT_sb[c][:, tb * 512:(tb + 1) * 512],
                                    start=(c == 0), stop=False)
                    c = NC_ - 1
                    for tb in range(4):
                        for f in range(2):
                            nc.tensor.matmul(
                                ps1[(f, tb)][:],
                                wqk_sb[c][:, f * 128:(f + 1) * 128],
                                xT_sb[c][:, tb * 512:(tb + 1) * 512],
                                start=False, stop=True)
                        for f in range(2):
                            dst = qk_sb[f][:, tb * 512:(tb + 1) * 512]
                            if f == 0:
                                nc.vector.tensor_copy(dst, ps1[(f, tb)][:])
                            else:
                                nc.scalar.copy(dst, ps1[(f, tb)][:])

                # ---- phase 2: attention per pair with PE fillers ----
                with tc.tile_pool(name="exp", bufs=4) as exp_pool, \
                     tc.tile_pool(name="rr", bufs=2) as r_pool, \
                     tc.tile_pool(name="rb", bufs=2) as rb_pool, \
                     tc.tile_pool(name="psS", bufs=2, space="PSUM") as psS, \
                     tc.tile_pool(name="psV", bufs=2, space="PSUM") as psV:

                    queue = FillQueue()

                    # one 2-bank "fill" PSUM pool serves every filler
                    # (v proj, f2/f3 proj, out-proj) across both pairs so
                    # leftovers can carry across the pair boundary
                    with tc.tile_pool(name="psFill", bufs=2,
                                      space="PSUM") as psF0:
                        # consumption-ordered interleave: the v tiles a
                        # q-block's AVs consume come ahead of the f2/f3
                        # blocks pair-1's same-numbered q-block needs
                        for tb in range(NQB):
                            for tt in range(tb * SJT, (tb + 1) * SJT):
                                queue.add(
                                    emit_v_group(tt, psF0, xT_sb, wv_sb, 2),
                                    key=("v", tt))
                            for f in (2, 3):
                                queue.add(
                                    emit_qk23_group(f, tb, psF0, xT_sb,
                                                    wqk_sb, 2),
                                    key=("qk23", f, tb))
                        # ~3.2 filler steps per k-tile slot balances the
                        # PE against the ~1.15us exp cadence across BOTH
                        # pairs; pair 0 deliberately leaves v/f23 leftovers
                        # to carry into pair 1 (whose own out-proj fillers
                        # only appear q-block by q-block)
                        attention_pair(0, psS, psV, r_pool, rb_pool,
                                       exp_pool, queue, spf=3.4,
                                       pull_min_jt=0, require_v=True)

                        def on_qb_done(qb, psF0=psF0, queue=queue):
                            # out-proj (both pairs accumulated) becomes
                            # filler as pair 1's q-blocks finish
                            ttp = QW // 128
                            for tt in range(qb * ttp, (qb + 1) * ttp):
                                queue.add(emit_outproj(tt, psF0))

                        def on_qb_start(qb, queue=queue):
                            # pair-1 q-block qb's scores read f2/f3 t-block
                            # qb; force those filler groups through first so
                            # the in-order PE queue can't deadlock
                            queue.require(("qk23", 3, qb))

                        # hold 3 out-proj groups in reserve past the loop;
                        # the drain spends them covering qb3's normalize-
                        # chain latency so the PE doesn't idle before the
                        # final out-proj tiles
                        queue.reserve_groups = 3
                        attention_pair(1, psS, psV, r_pool, rb_pool,
                                       exp_pool, queue, spf=2.4,
                                       on_qb_done=on_qb_done,
                                       on_qb_start=on_qb_start,
                                       pull_min_jt=2,
                                       terminal_last_qb=True)
                        queue.drain()

    nc.compile()
    return nc


def _prep_core_inputs(x, w_qkv, w_out, b, hg, nhc):
    """Per-core DRAM tensors for batch b, head-group hg."""
    Cc = x.shape[2]
    heads = [hg * nhc + i for i in range(nhc)]
    # wqk columns: per pair: [q_h0|q_h1] tile then [k_h0|k_h1] tile
    qk_rows = []
    for pair in range(nhc // 2):
        for qk in range(2):  # 0 = q, 1 = k
            for hh in range(2):
                hd = heads[pair * 2 + hh]
                qk_rows.append(w_qkv[qk * Cc + hd * 64:qk * Cc + (hd + 1) * 64, :])
    wqk_g = np.ascontiguousarray(np.concatenate(qk_rows, axis=0).T)
    v_rows = [w_qkv[2 * Cc + hd * 64:2 * Cc + (hd + 1) * 64, :] for hd in heads]
    wv_g = np.ascontiguousarray(np.concatenate(v_rows, axis=0).T)
    # wo rows ordered to match attn2 pair layout: pair p = heads (2p, 2p+1)
    wo_rows = [w_out[:, hd * 64:(hd + 1) * 64].T for hd in heads]
    wo_g = np.ascontiguousarray(np.concatenate(wo_rows, axis=0))
    return {
        "wqk": wqk_g.astype(np.float16),
        "wv": wv_g.astype(np.float16),
        "wo": wo_g.astype(np.float16),
    }


def _ensure_ntff_hook():
    """This image's antenv lacks axon_hooks; synthesize the module and
    register the ctypes NTFF profiling hook from trn_agent_boot so
    run_bass_kernel_spmd(trace=True) can capture HW exec time."""
    import types
    try:
        import antenv.axon_hooks  # noqa: F401
        return
    except ImportError:
        pass
    import antenv
    mod = types.ModuleType('antenv.axon_hooks')
    _h = {"hook": None}
    mod.set_axon_ntff_profile_hook = lambda h: _h.__setitem__("hook", h)
    mod.get_axon_ntff_profile_hook = lambda: _h["hook"]
    sys.modules['antenv.axon_hooks'] = mod
    antenv.axon_hooks = mod
    try:
        from trn_agent_boot.trn_boot import _ntff_profile_via_ctypes
        hook = _ntff_profile_via_ctypes('/opt/axon/libaxon_pjrt.so')
        if hook is not None:
            mod.set_axon_ntff_profile_hook(hook)
    except Exception:
        pass


def kernel(x, w_qkv, w_out):
    x = np.asarray(x, dtype=np.float32)
    w_qkv = np.asarray(w_qkv, dtype=np.float32)
    w_out = np.asarray(w_out, dtype=np.float32)

    key = "nc"
    if key not in _cache:
        _cache[key] = _build(T, C, HEADS_PER_CORE, D, N_CORES)
    nc = _cache[key]

    from concourse.bass_utils import run_bass_kernel_spmd

    if TRACE:
        _ensure_ntff_hook()

    tri = np.triu(np.ones((128, 128), dtype=np.float16))
    xTs = [np.ascontiguousarray(x[b].T.astype(np.float16)) for b in range(B)]
    in_maps = []
    for core in range(N_CORES):
        b, hg = core // 4, core % 4
        m = _prep_core_inputs(x, w_qkv, w_out, b, hg, HEADS_PER_CORE)
        m["xT"] = xTs[b]
        m["tri"] = tri
        in_maps.append(m)

    res = run_bass_kernel_spmd(nc, in_maps, core_ids=list(range(N_CORES)),
                               trace=TRACE)
    _cache["last_res"] = res
    partials = [res.results[i]["out"] for i in range(N_CORES)]
    out = np.empty((B, T, C), dtype=np.float32)
    for b in range(B):
        out[b] = np.sum(
            np.stack([partials[4 * b + j].astype(np.float32)
                      for j in range(4)]), axis=0)
    return out
